# revision 33
# baseline (speedup 1.0000x reference)
"""Trainium2 Bass kernel for BinaryTimedPSP (causal boxcar window sum + clip).

psp[t] = clip(sum_{k=max(0,t-D+1)}^{t} x[k], 0, 1) along time axis of a
[T=2048, B=16, N=2048] f32 spike tensor, D = duration (100).

Strategy: pure data-parallel over the 8 NeuronCores — the flattened B*N axis
(32768 columns) is split into 8 slabs of 4096 columns. Each core processes a
[T, 4096] slab.

Fast path (1 < D <= 129, i.e. the band spans 2 time chunks):
  - input spikes are exactly {0,1}; the host re-encodes them as fp8e4m3
    bytes (0x00/0x38) so the load traffic is 8 MiB/core instead of 32.
  - time tiled into 16 chunks of 128 rows; the window sum of chunk i is a
    block-banded matmul out_i = A_0 @ x_i + A_1 @ x_{i-1} with 0/1 weights.
    Both terms run as ONE fp8 DoubleRow matmul (lhsT [128,2,128] pairs
    subtile 0 = A_1^T with chunk i-1, subtile 1 = A_0^T with chunk i), and
    all fused matmuls share one stationary weight load.
  - all 16 x chunks live in SBUF at once ([128, 16, 4096] fp8 = 64 KiB per
    partition). w + chunk 0 go out on the scalar HWDGE queue and chunks
    1-3 on the gpsimd queue, in parallel with the bulk streaming as 1 MiB
    2-chunk batches on sync (each dma_start costs ~0.7us of issue time).
  - a few throwaway matmuls on a zeroed scratch tile run during the DMA
    preamble so the PE's HAM activity monitor lifts its 4/8 clock gate
    (1.2 -> 2.4 GHz) before the real matmuls begin.
  - clip to [0,1] == min(s,1) (sum >= 0) is split 4:4 across the two PSUM-
    capable engines as double-width [128,1024] ops draining 2 PSUM banks
    each: ftiles 0-3 on DVE as tensor_scalar_min, ftiles 4-7 on the scalar
    (ACT) engine as Relu(1 - s) — exact for integer s — which yields the
    COMPLEMENT; the host flips those columns back during the (lossless)
    fp8 -> f32 decode.
  - stores go out as 1 MiB chunk pairs on the gpsimd DMA queue (the engine
    is idle: no PSUM port, so it can't help with the clip), with the last
    four chunks stored singly so the tail drains behind the final clips.
Exactness: products/sums of 0/1 weights accumulate in f32 PSUM (<= 200),
fp8e4m3 holds {0,1} exactly, so the kernel is bit-exact vs the reference.
No cross-core communication; the gather is a host-side concatenate.
"""

import numpy as np

T_FULL, B_FULL, N_FULL = 2048, 16, 2048
NCORES = 8
P = 128
COLS = B_FULL * N_FULL          # 32768
FREE = COLS // NCORES           # 4096 columns per core
NCHUNK = T_FULL // P            # 16 time chunks
FTILE = 512                     # one PSUM bank of f32
NFT = FREE // FTILE             # 8
NFT_DVE = 4                     # ftiles 0-3 clipped on DVE (min), 4-7 on ACT

_CACHE: dict = {}


def _n_mats(d: int) -> int:
    # number of 128x128 band blocks: block m covers lags [128m-127, 128m+127]
    n = 1
    while P * n - (P - 1) <= d - 1 and n < NCHUNK:
        n += 1
    return n


def _weights(d: int, n_mats: int) -> np.ndarray:
    # W[m*128 + c, r] = A_m[r, c] = 1 iff 0 <= (r + 128m) - c < d
    # (lhsT layout: partition dim = contraction c, free dim = output row r)
    r = np.arange(P)[None, :]
    c = np.arange(P)[:, None]
    mats = []
    for m in range(n_mats):
        diff = r + P * m - c
        mats.append(((diff >= 0) & (diff < d)).astype(np.float32))
    return np.concatenate(mats, axis=0)


def _weights_fp8_pair(d: int) -> np.ndarray:
    """[128, 2*128] uint8 fp8 bytes: subtile 0 = A_1^T, subtile 1 = A_0^T."""
    w = _weights(d, 2)  # [2*128, 128] f32, block m at rows m*128
    pair = np.stack([w[P:], w[:P]], axis=1)  # [128, 2, 128], (A1, A0)
    return (pair.astype(np.uint8) * np.uint8(0x38)).reshape(P, 2 * P)


def _build_fast(d: int):
    """DoubleRow fp8 kernel, valid for 1 < d <= 129 (n_mats == 2)."""
    import concourse.bacc as bacc
    import concourse.mybir as mybir
    from concourse.tile import TileContext

    f32 = mybir.dt.float32
    f8 = mybir.dt.float8e4
    bf16 = mybir.dt.bfloat16
    relu = mybir.ActivationFunctionType.Relu
    drow = mybir.MatmulPerfMode.DoubleRow

    nc = bacc.Bacc(None)
    x = nc.dram_tensor("x", [T_FULL, FREE], f8, kind="ExternalInput")
    w = nc.dram_tensor("w", [P, 2 * P], f8, kind="ExternalInput")
    y = nc.dram_tensor("y", [T_FULL, FREE], f8, kind="ExternalOutput")
    # partition-major views so one DMA instruction can move 2 chunks
    xp = x.rearrange("(n p) f -> p n f", p=P)
    yp = y.rearrange("(n p) f -> p n f", p=P)
    wr = w.rearrange("p (k q) -> p k q", k=2)

    with nc.allow_low_precision("0/1 data; fp8e4 and f32 PSUM are exact"), \
         TileContext(nc) as tc:
        with (
            tc.tile_pool(name="wpool", bufs=1) as wpool,
            tc.tile_pool(name="xpool", bufs=1) as xpool,
            tc.tile_pool(name="opool", bufs=3) as opool,
            tc.tile_pool(name="ppool", bufs=4, space="PSUM") as ppool,
        ):
            xall = xpool.tile([P, NCHUNK, FREE], f8, tag="x")
            # head loads spread across three DMA queues so the first chunks
            # land while the sync ring is still issuing; the bulk streams as
            # 1 MiB 2-chunk loads on sync (batching halves the ~0.7us
            # per-dma_start issue cost)
            wt = wpool.tile([P, 2, P], f8, tag="w")
            # chunk 0 heads the fast sync ring (the scalar queue moves its
            # first transfers at only ~60 GB/s, which stalled the first
            # matmuls ~6us); w is tiny so the slow scalar queue is fine
            nc.scalar.dma_start(out=wt, in_=wr)
            nc.sync.dma_start(out=xall[:, 0:1, :], in_=xp[:, 0:1, :])
            nc.gpsimd.dma_start(out=xall[:, 1:2, :], in_=xp[:, 1:2, :])
            nc.gpsimd.dma_start(out=xall[:, 2:4, :], in_=xp[:, 2:4, :])
            for j in range(2, NCHUNK // 2):
                nc.sync.dma_start(
                    out=xall[:, 2 * j : 2 * j + 2, :], in_=xp[:, 2 * j : 2 * j + 2, :]
                )
            # HAM warm-up: throwaway matmuls on a zeroed scratch tile keep
            # the PE busy through the preamble AND through the wait for the
            # first x chunk, so the activity monitor lifts the 4/8 clock
            # gate before the real matmuls begin (and doesn't re-arm it)
            scr = wpool.tile([P, 2, FTILE], f8, tag="scr")
            nc.vector.memset(scr, 0)
            pwarm = ppool.tile([P, 2 * FTILE], f32, tag="ps")
            for _ in range(4):
                nc.tensor.matmul(
                    pwarm[:, :FTILE], wt, scr,
                    start=True, stop=True, perf_mode=drow,
                )
            for i in range(NCHUNK):
                if i % 2 == 0:
                    ot2 = opool.tile([P, 2, FREE], f8)
                ot = ot2[:, i % 2, :]
                # two PSUM banks per pool tile: two matmuls fill the halves,
                # then ONE double-width clip instruction drains both — halves
                # the DVE/ACT instruction + semaphore count
                for g in range(NFT // 2):
                    gs = slice(g * 2 * FTILE, (g + 1) * 2 * FTILE)
                    ps = ppool.tile([P, 2 * FTILE], f32, tag="ps")
                    for h in range(2):
                        f = 2 * g + h
                        fs = slice(f * FTILE, (f + 1) * FTILE)
                        hs = slice(h * FTILE, (h + 1) * FTILE)
                        if i == 0:
                            nc.tensor.matmul(
                                ps[:, hs], wt[:, 1, :], xall[:, 0, fs],
                                start=True, stop=True,
                            )
                        else:
                            nc.tensor.matmul(
                                ps[:, hs], wt, xall[:, i - 1 : i + 1, fs],
                                start=True, stop=True, perf_mode=drow,
                            )
                    if 2 * g < NFT_DVE:
                        nc.vector.tensor_scalar_min(
                            out=ot[:, gs], in0=ps, scalar1=1.0
                        )
                    else:
                        # exact complement: relu(1 - s) = 1 - min(s, 1)
                        nc.scalar.activation(
                            out=ot[:, gs], in_=ps, func=relu,
                            bias=1.0, scale=-1.0,
                        )
                # the store stream was the wall: 8.4 MB through the gpsimd
                # SWDGE queue runs at only ~180 GB/s (~48us). Split it:
                # first-half chunks go on gpsimd (they have slack), the
                # back half rides the fast sync HWDGE ring, which is idle
                # once the loads drain — singles/halves at the end so the
                # tail drains right behind the last clips
                if i < NCHUNK // 2:
                    if i % 2 == 1:
                        nc.gpsimd.dma_start(
                            out=yp[:, i - 1 : i + 1, :], in_=ot2
                        )
                elif i >= NCHUNK - 4:
                    if i == NCHUNK - 1:
                        half = FREE // 2
                        nc.sync.dma_start(out=yp[:, i, :half], in_=ot[:, :half])
                        nc.sync.dma_start(out=yp[:, i, half:], in_=ot[:, half:])
                    else:
                        nc.sync.dma_start(out=yp[:, i : i + 1, :], in_=ot2[:, i % 2 : i % 2 + 1, :])
                elif i % 2 == 1:
                    nc.sync.dma_start(
                        out=yp[:, i - 1 : i + 1, :], in_=ot2
                    )
    nc.finalize()
    return nc


def _build_generic(d: int):
    """f32r fallback (original baseline) for d outside the fast-path range."""
    import concourse.bacc as bacc
    import concourse.mybir as mybir
    from concourse.tile import TileContext

    n_mats = _n_mats(d)
    f32 = mybir.dt.float32
    f32r = mybir.dt.float32r
    f8 = mybir.dt.float8e4

    nc = bacc.Bacc(None)
    x = nc.dram_tensor("x", [T_FULL, FREE], f32r, kind="ExternalInput")
    w = nc.dram_tensor("w", [n_mats * P, P], f32r, kind="ExternalInput")
    y = nc.dram_tensor("y", [T_FULL, FREE], f8, kind="ExternalOutput")
    xr = x.rearrange("(n p) f -> n p f", p=P)
    yr = y.rearrange("(n p) f -> n p f", p=P)
    wr = w.rearrange("(m p) q -> m p q", p=P)

    with nc.allow_low_precision("output values are exactly 0/1; fp8e4 is lossless"), TileContext(nc) as tc:
        with (
            tc.tile_pool(name="wpool", bufs=1) as wpool,
            tc.tile_pool(name="xpool", bufs=4) as xpool,
            tc.tile_pool(name="opool", bufs=3) as opool,
            tc.tile_pool(name="ppool", bufs=8, space="PSUM") as ppool,
        ):
            xs = []
            x0 = xpool.tile([P, FREE], f32r, tag="x")
            nc.sync.dma_start(out=x0, in_=xr[0])
            xs.append(x0)
            wts = []
            for m in range(n_mats):
                wt = wpool.tile([P, P], f32r, tag=f"w{m}")
                nc.sync.dma_start(out=wt, in_=wr[m])
                wts.append(wt)
            for i in range(NCHUNK):
                if i > 0:
                    xt = xpool.tile([P, FREE], f32r, tag="x")
                    nc.sync.dma_start(out=xt, in_=xr[i])
                    xs.append(xt)
                ot = opool.tile([P, FREE], f8)
                terms = [m for m in range(n_mats) if i - m >= 0]
                for f in range(NFT):
                    ps = ppool.tile([P, FTILE], f32)
                    fs = slice(f * FTILE, (f + 1) * FTILE)
                    for j, m in enumerate(terms):
                        nc.tensor.matmul(
                            ps,
                            wts[m],
                            xs[i - m][:, fs],
                            start=(j == 0),
                            stop=(j == len(terms) - 1),
                        )
                    nc.vector.tensor_scalar_min(out=ot[:, fs], in0=ps, scalar1=1.0)
                if i == NCHUNK - 1:
                    half = FREE // 2
                    nc.scalar.dma_start(out=yr[i][:, :half], in_=ot[:, :half])
                    nc.scalar.dma_start(out=yr[i][:, half:], in_=ot[:, half:])
                else:
                    nc.scalar.dma_start(out=yr[i], in_=ot)
    nc.finalize()
    return nc


def _get_built(d: int):
    fast = 1 < d <= P + 1  # n_mats == 2
    if (d, fast) not in _CACHE:
        _CACHE[(d, fast)] = _build_fast(d) if fast else _build_generic(d)
    return _CACHE[(d, fast)], fast


def kernel(input_spikes, duration, _trace=False):
    import ml_dtypes
    from concourse.bass_utils import run_bass_kernel_spmd

    x = np.asarray(input_spikes)
    d = int(duration)
    assert x.shape == (T_FULL, B_FULL, N_FULL), x.shape

    nc, fast = _get_built(d)

    if fast:
        # exact re-encode of {0,1} f32 -> fp8e4m3 bytes
        xf8 = (
            (x.reshape(T_FULL, COLS) != 0).astype(np.uint8) * np.uint8(0x38)
        ).view(ml_dtypes.float8_e4m3)
        W = _weights_fp8_pair(d).view(ml_dtypes.float8_e4m3)
        in_maps = [
            {"x": np.ascontiguousarray(xf8[:, c * FREE : (c + 1) * FREE]), "w": W}
            for c in range(NCORES)
        ]
    else:
        xf = np.ascontiguousarray(x.astype(np.float32)).reshape(T_FULL, COLS)
        W = _weights(d, _n_mats(d))
        in_maps = [
            {"x": np.ascontiguousarray(xf[:, c * FREE : (c + 1) * FREE]), "w": W}
            for c in range(NCORES)
        ]

    res = run_bass_kernel_spmd(
        nc, in_maps, core_ids=list(range(NCORES)), trace=_trace
    )
    out = np.concatenate([r["y"] for r in res.results], axis=1)
    out = out.astype(np.float32)
    if fast:
        # ftiles NFT_DVE.. of each core slab hold the complement (ACT path)
        o4 = out.reshape(T_FULL, NCORES, NFT, FTILE)
        o4[:, :, NFT_DVE:, :] = 1.0 - o4[:, :, NFT_DVE:, :]
    out = out.reshape(T_FULL, B_FULL, N_FULL)
    if _trace:
        return out, res
    return out
